# revision 18
# baseline (speedup 1.0000x reference)
"""Causal self-attention (GQA + RoPE) Trainium2 Bass kernel, 8 NeuronCores.

Sharding: tensor-parallel over head groups (4 groups x 8 q heads, each with
its 2 kv heads) x data-parallel over batch (2). Core c handles batch c//4,
head group c%4. After attention, yT ([feat, T]) is AllGather'd over each
4-core group (partition-axis concat) in 4 t-chunks (overlapped with
compute), and each core computes a 512-wide output-column slice of the
final projection, so no all-reduce is needed. The host assembles the full
output from the 8 column slices.

Kernel math per core (g = c%4, b = c//4), bf16 matmuls / fp32 accumulation:
  q,kv  = x[b] @ [Wq|Wk|Wv]_g.T   (pre-transposed operands, 2 MMs per chunk)
  q,k   = rope(...); PE-transpose to qT/kT ([d, t], head-dim on partitions)
  sT    = kT.T @ qT per (s-tile, t-chunk)   -> scores already transposed
  attT  = exp(sT * 0.125 + causal_mask)     (no max subtraction: |logit|<~40)
  yT_aug= vaug.T @ attT  (vaug = [v | ones] -> row 64 = softmax denominators)
  yT    = yT_aug[0:64] * recip(yT_aug[64])  -> AllGather -> proj slice.
"""

import numpy as np
import ml_dtypes
from contextlib import ExitStack

import concourse.bass as bass
import concourse.tile as tile
from concourse import bacc, mybir
from concourse.bass import ts, ds
from concourse.bass_utils import run_bass_kernel_spmd
from concourse.masks import make_identity

F32 = mybir.dt.float32
BF16 = mybir.dt.bfloat16

# compute dtype knobs
DT = BF16                  # matmul operand dtype (x, W, qT/kT, attT, v, yT)
NP_DT = ml_dtypes.bfloat16 if DT == BF16 else np.float32

N_CORES = 8
T = 2048
C = 2048
HD = 64
N_HEAD = 32
N_KV = 8
NH_LOC = 8
QW = NH_LOC * HD          # 512
KW = 2 * HD               # 128 (2 kv heads)
VAW = HD + 1              # v plus ones column
EW = 512                  # output-column slice per core
P = 128
TT = T // P               # 16 t-tiles
TC = 4                    # t-chunks (512 wide) for attention + AG + proj
KC = C // P               # 16 contraction chunks
SCALE = 1.0 / 8.0
NEG = -1.0e30

REPLICA_GROUPS = [[0, 1, 2, 3], [4, 5, 6, 7]]

_CACHE = {}


def build_program():
    nc = bacc.Bacc("TRN2", target_bir_lowering=False, debug=False,
                   enable_asserts=False, num_devices=N_CORES)

    xT = nc.dram_tensor("xT", [C, T], DT, kind="ExternalInput").ap()
    wqkvT = nc.dram_tensor("wqkvT", [C, QW + 2 * KW], DT,
                           kind="ExternalInput").ap()
    wpT = nc.dram_tensor("wpT", [C, EW], DT, kind="ExternalInput").ap()
    cosf = nc.dram_tensor("cosf", [T, HD // 2], F32, kind="ExternalInput").ap()
    sinf = nc.dram_tensor("sinf", [T, HD // 2], F32, kind="ExternalInput").ap()
    out = nc.dram_tensor("out", [T, EW], F32, kind="ExternalOutput").ap()

    with tile.TileContext(nc) as tc, ExitStack() as ctx:
        sb = ctx.enter_context(tc.tile_pool(name="sb", bufs=1))
        work = ctx.enter_context(tc.tile_pool(name="work", bufs=2))
        psum = ctx.enter_context(tc.tile_pool(name="psum", bufs=1, space="PSUM"))
        dram = ctx.enter_context(tc.tile_pool(name="dram", bufs=1, space="DRAM"))

        # ---- static tiles -------------------------------------------------
        ident = sb.tile([P, P], DT)
        make_identity(nc, ident)
        ones64 = sb.tile([1, 64], F32)
        nc.vector.memset(ones64[:], 1.0)
        # sT-orientation causal mask for a diagonal block [s, t]:
        # keep (0) where s <= t i.e. col >= row, else NEG
        cmaskT = sb.tile([P, P], F32)
        nc.gpsimd.memset(cmaskT[:], 0.0)
        nc.gpsimd.affine_select(
            out=cmaskT[:], in_=cmaskT[:], compare_op=mybir.AluOpType.is_ge,
            fill=NEG, base=0, pattern=[[1, P]], channel_multiplier=-1)

        # ---- resident weights / tables -----------------------------------
        # per-chunk loads so the first matmuls start after ~0.4MB, not 6MB
        wqkv_sb = sb.tile([P, KC, QW + 2 * KW], DT)
        wqkv_r = wqkvT.rearrange("(ko ki) j -> ki ko j", ki=P)
        for kc in range(KC):
            nc.sync.dma_start(wqkv_sb[:, kc], wqkv_r[:, kc])
        wp_sb = sb.tile([P, KC, EW], DT)
        wp_r = wpT.rearrange("(ko ki) j -> ki ko j", ki=P)
        for kc in range(KC):
            nc.sync.dma_start(wp_sb[:, kc], wp_r[:, kc])
        cos_sb = sb.tile([P, TT, HD // 2], F32)
        nc.sync.dma_start(cos_sb[:], cosf.rearrange("(to ti) f -> ti to f", ti=P))
        sin_sb = sb.tile([P, TT, HD // 2], F32)
        nc.sync.dma_start(sin_sb[:], sinf.rearrange("(to ti) f -> ti to f", ti=P))

        # ---- persistent activations --------------------------------------
        qT_store = sb.tile([P, NH_LOC // 2, T], DT)  # pair p, head parity halves
        kT_rep = sb.tile([P, 2, T], DT)              # kv head kvl, replicated
        vaug = sb.tile([P, TT, 2, VAW], DT)          # [t, tile, kvl, d|1]
        nc.vector.memset(vaug[:], 1.0)               # ones column (rest overwritten)

        # ---- DRAM intermediates ------------------------------------------
        yT_loc = [dram.tile([QW, 512], DT, name=f"yT_loc{m}") for m in range(TC)]
        yT_full = [dram.tile([4 * QW, 512], DT, name=f"yT_full{m}")
                   for m in range(TC)]

        def rope(dst, src, n_heads, ti):
            """src: SBUF [128, n_heads*64] (so gpsimd can help); dst same."""
            sv = src.rearrange("p (h f two) -> p h f two", h=n_heads, two=2)
            dv = dst.rearrange("p (h f two) -> p h f two", h=n_heads, two=2)
            cb = cos_sb[:, ti, None, :].to_broadcast([P, n_heads, HD // 2])
            sbr = sin_sb[:, ti, None, :].to_broadcast([P, n_heads, HD // 2])
            x0, x1 = sv[..., 0], sv[..., 1]
            t0 = work.tile([P, n_heads, HD // 2], F32, tag=f"rope_t0_{n_heads}")
            t1 = work.tile([P, n_heads, HD // 2], F32, tag=f"rope_t1_{n_heads}")
            nc.vector.tensor_tensor(t0[:], x1, sbr, mybir.AluOpType.mult)
            nc.vector.tensor_tensor(t1[:], x1, cb, mybir.AluOpType.mult)
            nc.vector.tensor_tensor(dv[..., 0], x0, cb, mybir.AluOpType.mult)
            nc.vector.tensor_tensor(dv[..., 0], dv[..., 0], t0[:],
                                    mybir.AluOpType.subtract)
            nc.vector.tensor_tensor(dv[..., 1], x0, sbr, mybir.AluOpType.mult)
            nc.vector.tensor_tensor(dv[..., 1], dv[..., 1], t1[:],
                                    mybir.AluOpType.add)

        # ================= phase A: qkv + rope + transposes ================
        def a_chunk(m4):
            # one 2MB x-load per 4 t-tiles: 1KB DMA lines instead of 256B
            xt4 = work.tile([P, KC, 512], DT, tag="xt4")
            nc.sync.dma_start(
                xt4[:],
                xT[:, ts(m4, 512)].rearrange("(ko ki) t -> ki ko t", ki=P))
            for it in range(4):
                i = 4 * m4 + it
                xt = xt4[:, :, ts(it, P)]

                q_ps = psum.tile([P, QW], F32, tag="mm512", bufs=3)
                kv_ps = psum.tile([P, 2 * KW], F32, tag="small", bufs=2)
                for kc in range(KC):
                    nc.tensor.matmul(q_ps[:], xt[:, kc], wqkv_sb[:, kc, :QW],
                                     start=(kc == 0), stop=(kc == KC - 1))
                for kc in range(KC):
                    nc.tensor.matmul(kv_ps[:], xt[:, kc], wqkv_sb[:, kc, QW:],
                                     start=(kc == 0), stop=(kc == KC - 1))

                for kvl in range(2):
                    nc.scalar.copy(vaug[:, i, kvl, :HD],
                                   kv_ps[:, ds(KW + kvl * HD, HD)])

                # stage q/k to SBUF first: frees PSUM fast and lets gpsimd
                # (no-PSUM engine) take part of the rope arithmetic
                q_sb = work.tile([P, QW], DT, tag="q_sb")
                nc.scalar.copy(q_sb[:], q_ps[:])
                k_sb = work.tile([P, KW], DT, tag="k_sb")
                nc.scalar.copy(k_sb[:], kv_ps[:, :KW])
                q_rope = work.tile([P, QW], DT, tag="q_rope")
                rope(q_rope, q_sb, NH_LOC, i)
                k_rope = work.tile([P, KW], DT, tag="k_rope")
                rope(k_rope, k_sb, 2, i)

                for p in range(NH_LOC // 2):
                    tp = psum.tile([P, P], DT, tag="small", bufs=2, name="tp_q")
                    nc.tensor.transpose(tp[:], q_rope[:, ts(p, P)], ident[:])
                    nc.scalar.copy(qT_store[:, p, ts(i, P)], tp[:])

                tpk = psum.tile([P, P], DT, tag="small", bufs=2, name="tp_k")
                nc.tensor.transpose(tpk[:], k_rope[:], ident[:])
                ktt = work.tile([P, P], DT, tag="ktt")
                nc.scalar.copy(ktt[:], tpk[:])
                nc.scalar.copy(kT_rep[0:64, 0, ts(i, P)], ktt[0:64])
                nc.scalar.copy(kT_rep[64:128, 1, ts(i, P)], ktt[64:128])
                nc.sync.dma_start(kT_rep[64:128, 0, ts(i, P)], ktt[0:64])
                nc.sync.dma_start(kT_rep[0:64, 1, ts(i, P)], ktt[64:128])

        # ============ phase B/C: attention + normalize + AG (per t-chunk) ==
        def b_chunk(m):
            n_j = 4 * m + 4          # s-tiles 0..4m+3
            for pr in range(NH_LOC // 2):
                # heads h0 = 2*pr (partitions 0:64), h1 = 2*pr+1 (64:128):
                # adjacent K=64 score MMs land in distinct PE row groups and
                # run concurrently; the av MMs share one stationary operand.
                kvl = pr // 2
                y_ps = [psum.tile([P, 512], F32, tag="yaug", bufs=3,
                                  name=f"y_ps{h}") for h in range(2)]
                for j in range(n_j):
                    off = max(0, j * P - m * 512)
                    cw = 512 - off
                    ps_s = [psum.tile([P, 512], F32, tag="mm512", bufs=3,
                                      name=f"ps_s{h}") for h in range(2)]
                    for h in range(2):
                        lo = h * 64
                        nc.tensor.matmul(
                            ps_s[h][:, ds(off, cw)],
                            kT_rep[lo:lo + 64, kvl, ts(j, P)],
                            qT_store[lo:lo + 64, pr, ds(m * 512 + off, cw)],
                            start=True, stop=True)
                    attT = [None, None]
                    for h in range(2):
                        if j * P >= m * 512:  # diagonal block: causal mask
                            nc.vector.tensor_tensor(
                                ps_s[h][:, ds(off, P)], ps_s[h][:, ds(off, P)],
                                cmaskT[:], mybir.AluOpType.add)
                        attT[h] = work.tile([P, 512], DT, tag="attT", bufs=4,
                                            name=f"attT{h}")
                        nc.scalar.activation(
                            attT[h][:, :cw], ps_s[h][:, ds(off, cw)],
                            mybir.ActivationFunctionType.Exp,
                            bias=0.0, scale=SCALE)
                    for h in range(2):
                        nc.tensor.matmul(
                            y_ps[h][:VAW, ds(off, cw)],
                            vaug[:, j, kvl], attT[h][:, :cw],
                            start=(j == 0), stop=(j == n_j - 1))
                for h in range(2):
                    # quick-release: two copies free the accumulator PSUM,
                    # then normalize entirely from SBUF
                    yta = work.tile([64, 512], F32, tag="yta")
                    nc.vector.tensor_copy(yta[:], y_ps[h][:64, :])
                    srow = work.tile([1, 512], F32, tag="srow")
                    nc.scalar.copy(srow[:], y_ps[h][64:65, :])
                    sbc = psum.tile([64, 512], F32, tag="small", bufs=2,
                                    name="sbc")
                    nc.tensor.matmul(sbc[:], ones64[:], srow[:],
                                     start=True, stop=True)
                    rbc = work.tile([64, 512], F32, tag="rbc")
                    nc.vector.reciprocal_approx_fast(rbc[:], sbc[:])
                    ytn = work.tile([64, 512], DT, tag="ytn")
                    nc.vector.tensor_tensor(ytn[:], yta[:], rbc[:],
                                            mybir.AluOpType.mult)
                    nc.sync.dma_start(yT_loc[m][ts(2 * pr + h, HD), :], ytn[:])

            nc.gpsimd.collective_compute(
                "AllGather", mybir.AluOpType.bypass,
                replica_groups=REPLICA_GROUPS,
                ins=[yT_loc[m].opt()], outs=[yT_full[m].opt()])

        # Interleave qkv chunks with attention chunks in program order: the
        # scheduler then has dense independent PE work to fill the per-step
        # score->exp->av dependency bubbles (keeps the PE HAM clock at 2.4GHz)
        a_chunk(0)
        a_chunk(1)
        b_chunk(0)
        a_chunk(2)
        b_chunk(1)
        a_chunk(3)
        b_chunk(2)
        b_chunk(3)

        # ================= phase D: output projection ======================
        for m in range(TC):
            yt4 = work.tile([P, KC, 512], DT, tag="xt4", name="yt4")
            nc.sync.dma_start(
                yt4[:], yT_full[m].rearrange("(ko ki) t -> ki ko t", ki=P))
            for it in range(4):
                yt = yt4[:, :, ts(it, P)]
                o_ps = psum.tile([P, EW], F32, tag="mm512", bufs=3, name="o_ps")
                for kc in range(KC):
                    nc.tensor.matmul(o_ps[:], yt[:, kc], wp_sb[:, kc],
                                     start=(kc == 0), stop=(kc == KC - 1))
                o_sb = work.tile([P, EW], F32, tag="o_sb")
                nc.vector.tensor_copy(o_sb[:], o_ps[:])
                nc.sync.dma_start(out[ts(4 * m + it, P), :], o_sb[:])

    nc.compile()
    return nc


def _get_program():
    if "nc" not in _CACHE:
        _CACHE["nc"] = build_program()
    return _CACHE["nc"]


def prepare_in_maps(x, Wqkv, Wproj, freqs_cos, freqs_sin):
    x = np.asarray(x, dtype=np.float32)
    Wqkv = np.asarray(Wqkv, dtype=np.float32)
    Wproj = np.asarray(Wproj, dtype=np.float32)
    cosf = np.ascontiguousarray(np.asarray(freqs_cos, dtype=np.float32))
    sinf = np.ascontiguousarray(np.asarray(freqs_sin, dtype=np.float32))

    Wq = Wqkv[:N_HEAD * HD]
    Wk = Wqkv[N_HEAD * HD:(N_HEAD + N_KV) * HD]
    Wv = Wqkv[(N_HEAD + N_KV) * HD:]

    in_maps = []
    for c in range(N_CORES):
        g, b = c % 4, c // 4
        xTb = np.ascontiguousarray(x[b].T.astype(NP_DT))
        wq = Wq[g * QW:(g + 1) * QW]
        wk = Wk[g * KW:(g + 1) * KW]
        wv = Wv[g * KW:(g + 1) * KW]
        wqkvT = np.ascontiguousarray(
            np.concatenate([wq, wk, wv], axis=0).T.astype(NP_DT))
        wpT = np.ascontiguousarray(Wproj[g * EW:(g + 1) * EW].T.astype(NP_DT))
        in_maps.append({"xT": xTb, "wqkvT": wqkvT, "wpT": wpT,
                        "cosf": cosf, "sinf": sinf})
    return in_maps


def run(in_maps, **kw):
    nc = _get_program()
    return run_bass_kernel_spmd(nc, in_maps, core_ids=list(range(N_CORES)), **kw)


def assemble(results):
    B = 2
    out = np.empty((B, T, C), dtype=np.float32)
    for c in range(N_CORES):
        g, b = c % 4, c // 4
        out[b][:, g * EW:(g + 1) * EW] = results[c]["out"]
    return out


def kernel(x, Wqkv, Wproj, freqs_cos, freqs_sin, start_pos=None, **_ignored):
    in_maps = prepare_in_maps(x, Wqkv, Wproj, freqs_cos, freqs_sin)
    res = run(in_maps)
    return assemble(res.results)


# revision 19
# speedup vs baseline: 1.0928x; 1.0928x over previous
"""Causal self-attention (GQA + RoPE) Trainium2 Bass kernel, 8 NeuronCores.

Sharding: tensor-parallel over head groups (4 groups x 8 q heads, each with
its 2 kv heads) x data-parallel over batch (2). Core c handles batch c//4,
head group c%4. After attention, yT ([feat, T]) is AllGather'd over each
4-core group (partition-axis concat) in 4 t-chunks (overlapped with
compute), and each core computes a 512-wide output-column slice of the
final projection, so no all-reduce is needed. The host assembles the full
output from the 8 column slices.

Kernel math per core (g = c%4, b = c//4), bf16 matmuls / fp32 accumulation:
  q,kv  = x[b] @ [Wq|Wk|Wv]_g.T   (pre-transposed operands, 2 MMs per chunk)
  q,k   = rope(...); PE-transpose to qT/kT ([d, t], head-dim on partitions)
  sT    = kT.T @ qT per (s-tile, t-chunk)   -> scores already transposed
  attT  = exp(sT * 0.125 + causal_mask)     (no max subtraction: |logit|<~40)
  yT_aug= vaug.T @ attT  (vaug = [v | ones] -> row 64 = softmax denominators)
  yT    = yT_aug[0:64] * recip(yT_aug[64])  -> AllGather -> proj slice.
"""

import numpy as np
import ml_dtypes
from contextlib import ExitStack

import concourse.bass as bass
import concourse.tile as tile
from concourse import bacc, mybir
from concourse.bass import ts, ds
from concourse.bass_utils import run_bass_kernel_spmd
from concourse.masks import make_identity

F32 = mybir.dt.float32
BF16 = mybir.dt.bfloat16

# compute dtype knobs
DT = BF16                  # matmul operand dtype (x, W, qT/kT, attT, v, yT)
NP_DT = ml_dtypes.bfloat16 if DT == BF16 else np.float32

N_CORES = 8
T = 2048
C = 2048
HD = 64
N_HEAD = 32
N_KV = 8
NH_LOC = 8
QW = NH_LOC * HD          # 512
KW = 2 * HD               # 128 (2 kv heads)
VAW = HD + 1              # v plus ones column
EW = 512                  # output-column slice per core
P = 128
TT = T // P               # 16 t-tiles
TC = 4                    # t-chunks (512 wide) for attention + AG + proj
KC = C // P               # 16 contraction chunks
SCALE = 1.0 / 8.0
NEG = -1.0e30

REPLICA_GROUPS = [[0, 1, 2, 3], [4, 5, 6, 7]]

_CACHE = {}


def build_program():
    nc = bacc.Bacc("TRN2", target_bir_lowering=False, debug=False,
                   enable_asserts=False, num_devices=N_CORES)

    xT = nc.dram_tensor("xT", [C, T], DT, kind="ExternalInput").ap()
    wqkvT = nc.dram_tensor("wqkvT", [C, QW + 2 * KW], DT,
                           kind="ExternalInput").ap()
    wpT = nc.dram_tensor("wpT", [C, EW], DT, kind="ExternalInput").ap()
    cosf = nc.dram_tensor("cosf", [T, HD // 2], F32, kind="ExternalInput").ap()
    sinf = nc.dram_tensor("sinf", [T, HD // 2], F32, kind="ExternalInput").ap()
    out = nc.dram_tensor("out", [T, EW], F32, kind="ExternalOutput").ap()

    with tile.TileContext(nc) as tc, ExitStack() as ctx:
        sb = ctx.enter_context(tc.tile_pool(name="sb", bufs=1))
        work = ctx.enter_context(tc.tile_pool(name="work", bufs=2))
        psum = ctx.enter_context(tc.tile_pool(name="psum", bufs=1, space="PSUM"))
        dram = ctx.enter_context(tc.tile_pool(name="dram", bufs=1, space="DRAM"))

        # ---- static tiles -------------------------------------------------
        ident = sb.tile([P, P], DT)
        make_identity(nc, ident)
        ones64 = sb.tile([1, 64], F32)
        nc.vector.memset(ones64[:], 1.0)
        # sT-orientation causal mask for a diagonal block [s, t]:
        # keep (0) where s <= t i.e. col >= row, else NEG
        cmaskT = sb.tile([P, P], F32)
        nc.gpsimd.memset(cmaskT[:], 0.0)
        nc.gpsimd.affine_select(
            out=cmaskT[:], in_=cmaskT[:], compare_op=mybir.AluOpType.is_ge,
            fill=NEG, base=0, pattern=[[1, P]], channel_multiplier=-1)

        # ---- resident weights / tables -----------------------------------
        # per-chunk loads so the first matmuls start after ~0.4MB, not 6MB
        wqkv_sb = sb.tile([P, KC, QW + 2 * KW], DT)
        wqkv_r = wqkvT.rearrange("(ko ki) j -> ki ko j", ki=P)
        for kc in range(KC):
            nc.sync.dma_start(wqkv_sb[:, kc], wqkv_r[:, kc])
        wp_sb = sb.tile([P, KC, EW], DT)
        wp_r = wpT.rearrange("(ko ki) j -> ki ko j", ki=P)
        for kc in range(KC):
            nc.sync.dma_start(wp_sb[:, kc], wp_r[:, kc])
        cos_sb = sb.tile([P, TT, HD // 2], F32)
        nc.sync.dma_start(cos_sb[:], cosf.rearrange("(to ti) f -> ti to f", ti=P))
        sin_sb = sb.tile([P, TT, HD // 2], F32)
        nc.sync.dma_start(sin_sb[:], sinf.rearrange("(to ti) f -> ti to f", ti=P))

        # ---- persistent activations --------------------------------------
        qT_store = sb.tile([P, NH_LOC // 2, T], DT)  # pair p, head parity halves
        kT_rep = sb.tile([P, 2, T], DT)              # kv head kvl, replicated
        vaug = sb.tile([P, TT, 2, VAW], DT)          # [t, tile, kvl, d|1]
        nc.vector.memset(vaug[:], 1.0)               # ones column (rest overwritten)

        # ---- DRAM intermediates ------------------------------------------
        yT_loc = [dram.tile([QW, 512], DT, name=f"yT_loc{m}") for m in range(TC)]
        yT_full = [dram.tile([4 * QW, 512], DT, name=f"yT_full{m}")
                   for m in range(TC)]

        def rope(dst, src, n_heads, ti):
            """src: SBUF [128, n_heads*64] (so gpsimd can help); dst same."""
            sv = src.rearrange("p (h f two) -> p h f two", h=n_heads, two=2)
            dv = dst.rearrange("p (h f two) -> p h f two", h=n_heads, two=2)
            cb = cos_sb[:, ti, None, :].to_broadcast([P, n_heads, HD // 2])
            sbr = sin_sb[:, ti, None, :].to_broadcast([P, n_heads, HD // 2])
            x0, x1 = sv[..., 0], sv[..., 1]
            t0 = work.tile([P, n_heads, HD // 2], F32, tag=f"rope_t0_{n_heads}")
            t1 = work.tile([P, n_heads, HD // 2], F32, tag=f"rope_t1_{n_heads}")
            nc.vector.tensor_tensor(t0[:], x1, sbr, mybir.AluOpType.mult)
            nc.vector.tensor_tensor(t1[:], x1, cb, mybir.AluOpType.mult)
            nc.vector.tensor_tensor(dv[..., 0], x0, cb, mybir.AluOpType.mult)
            nc.vector.tensor_tensor(dv[..., 0], dv[..., 0], t0[:],
                                    mybir.AluOpType.subtract)
            nc.vector.tensor_tensor(dv[..., 1], x0, sbr, mybir.AluOpType.mult)
            nc.vector.tensor_tensor(dv[..., 1], dv[..., 1], t1[:],
                                    mybir.AluOpType.add)

        # ================= phase A: qkv + rope + transposes ================
        def a_chunk(m4):
            # one 2MB x-load per 4 t-tiles: 1KB DMA lines instead of 256B
            xt4 = work.tile([P, KC, 512], DT, tag="xt4")
            nc.sync.dma_start(
                xt4[:],
                xT[:, ts(m4, 512)].rearrange("(ko ki) t -> ki ko t", ki=P))
            for it in range(4):
                i = 4 * m4 + it
                xt = xt4[:, :, ts(it, P)]

                q_ps = psum.tile([P, QW], F32, tag="mm512", bufs=3)
                kv_ps = psum.tile([P, 2 * KW], F32, tag="small", bufs=2)
                for kc in range(KC):
                    nc.tensor.matmul(q_ps[:], xt[:, kc], wqkv_sb[:, kc, :QW],
                                     start=(kc == 0), stop=(kc == KC - 1))
                for kc in range(KC):
                    nc.tensor.matmul(kv_ps[:], xt[:, kc], wqkv_sb[:, kc, QW:],
                                     start=(kc == 0), stop=(kc == KC - 1))

                for kvl in range(2):
                    nc.scalar.copy(vaug[:, i, kvl, :HD],
                                   kv_ps[:, ds(KW + kvl * HD, HD)])

                # stage q/k to SBUF first: frees PSUM fast and lets gpsimd
                # (no-PSUM engine) take part of the rope arithmetic
                q_sb = work.tile([P, QW], DT, tag="q_sb")
                nc.scalar.copy(q_sb[:], q_ps[:])
                k_sb = work.tile([P, KW], DT, tag="k_sb")
                nc.scalar.copy(k_sb[:], kv_ps[:, :KW])
                q_rope = work.tile([P, QW], DT, tag="q_rope")
                rope(q_rope, q_sb, NH_LOC, i)
                k_rope = work.tile([P, KW], DT, tag="k_rope")
                rope(k_rope, k_sb, 2, i)

                for p in range(NH_LOC // 2):
                    tp = psum.tile([P, P], DT, tag="small", bufs=2, name="tp_q")
                    nc.tensor.transpose(tp[:], q_rope[:, ts(p, P)], ident[:])
                    nc.scalar.copy(qT_store[:, p, ts(i, P)], tp[:])

                tpk = psum.tile([P, P], DT, tag="small", bufs=2, name="tp_k")
                nc.tensor.transpose(tpk[:], k_rope[:], ident[:])
                ktt = work.tile([P, P], DT, tag="ktt")
                nc.scalar.copy(ktt[:], tpk[:])
                nc.scalar.copy(kT_rep[0:64, 0, ts(i, P)], ktt[0:64])
                nc.scalar.copy(kT_rep[64:128, 1, ts(i, P)], ktt[64:128])
                nc.sync.dma_start(kT_rep[64:128, 0, ts(i, P)], ktt[0:64])
                nc.sync.dma_start(kT_rep[0:64, 1, ts(i, P)], ktt[64:128])

        # ============ phase B/C: attention + normalize + AG (per t-chunk) ==
        def b_chunk(m):
            n_j = 4 * m + 4          # s-tiles 0..4m+3
            for pr in range(NH_LOC // 2):
                # heads h0 = 2*pr (partitions 0:64), h1 = 2*pr+1 (64:128):
                # adjacent K=64 score MMs land in distinct PE row groups and
                # run concurrently; the av MMs share one stationary operand.
                kvl = pr // 2
                y_ps = [psum.tile([P, 512], F32, tag="yaug", bufs=3,
                                  name=f"y_ps{h}") for h in range(2)]
                for j in range(n_j):
                    off = max(0, j * P - m * 512)
                    cw = 512 - off
                    ps_s = [psum.tile([P, 512], F32, tag="mm512", bufs=3,
                                      name=f"ps_s{h}") for h in range(2)]
                    for h in range(2):
                        lo = h * 64
                        nc.tensor.matmul(
                            ps_s[h][:, ds(off, cw)],
                            kT_rep[lo:lo + 64, kvl, ts(j, P)],
                            qT_store[lo:lo + 64, pr, ds(m * 512 + off, cw)],
                            start=True, stop=True)
                    attT = [None, None]
                    for h in range(2):
                        if j * P >= m * 512:  # diagonal block: causal mask
                            nc.vector.tensor_tensor(
                                ps_s[h][:, ds(off, P)], ps_s[h][:, ds(off, P)],
                                cmaskT[:], mybir.AluOpType.add)
                        attT[h] = work.tile([P, 512], DT, tag="attT", bufs=6,
                                            name=f"attT{h}")
                        nc.scalar.activation(
                            attT[h][:, :cw], ps_s[h][:, ds(off, cw)],
                            mybir.ActivationFunctionType.Exp,
                            bias=0.0, scale=SCALE)
                    for h in range(2):
                        nc.tensor.matmul(
                            y_ps[h][:VAW, ds(off, cw)],
                            vaug[:, j, kvl], attT[h][:, :cw],
                            start=(j == 0), stop=(j == n_j - 1))
                for h in range(2):
                    # quick-release: two copies free the accumulator PSUM,
                    # then normalize entirely from SBUF
                    yta = work.tile([64, 512], F32, tag="yta")
                    nc.vector.tensor_copy(yta[:], y_ps[h][:64, :])
                    srow = work.tile([1, 512], F32, tag="srow")
                    nc.scalar.copy(srow[:], y_ps[h][64:65, :])
                    sbc = psum.tile([64, 512], F32, tag="small", bufs=2,
                                    name="sbc")
                    nc.tensor.matmul(sbc[:], ones64[:], srow[:],
                                     start=True, stop=True)
                    rbc = work.tile([64, 512], F32, tag="rbc")
                    nc.vector.reciprocal_approx_fast(rbc[:], sbc[:])
                    ytn = work.tile([64, 512], DT, tag="ytn")
                    nc.vector.tensor_tensor(ytn[:], yta[:], rbc[:],
                                            mybir.AluOpType.mult)
                    nc.sync.dma_start(yT_loc[m][ts(2 * pr + h, HD), :], ytn[:])

            nc.gpsimd.collective_compute(
                "AllGather", mybir.AluOpType.bypass,
                replica_groups=REPLICA_GROUPS,
                ins=[yT_loc[m].opt()], outs=[yT_full[m].opt()])

        for _m in range(TC):
            a_chunk(_m)
        for _m in range(TC):
            b_chunk(_m)

        # ================= phase D: output projection ======================
        for m in range(TC):
            yt4 = work.tile([P, KC, 512], DT, tag="xt4", name="yt4")
            nc.sync.dma_start(
                yt4[:], yT_full[m].rearrange("(ko ki) t -> ki ko t", ki=P))
            for it in range(4):
                yt = yt4[:, :, ts(it, P)]
                o_ps = psum.tile([P, EW], F32, tag="mm512", bufs=3, name="o_ps")
                for kc in range(KC):
                    nc.tensor.matmul(o_ps[:], yt[:, kc], wp_sb[:, kc],
                                     start=(kc == 0), stop=(kc == KC - 1))
                o_sb = work.tile([P, EW], F32, tag="o_sb")
                nc.vector.tensor_copy(o_sb[:], o_ps[:])
                nc.sync.dma_start(out[ts(4 * m + it, P), :], o_sb[:])

    nc.compile()
    return nc


def _get_program():
    if "nc" not in _CACHE:
        _CACHE["nc"] = build_program()
    return _CACHE["nc"]


def prepare_in_maps(x, Wqkv, Wproj, freqs_cos, freqs_sin):
    x = np.asarray(x, dtype=np.float32)
    Wqkv = np.asarray(Wqkv, dtype=np.float32)
    Wproj = np.asarray(Wproj, dtype=np.float32)
    cosf = np.ascontiguousarray(np.asarray(freqs_cos, dtype=np.float32))
    sinf = np.ascontiguousarray(np.asarray(freqs_sin, dtype=np.float32))

    Wq = Wqkv[:N_HEAD * HD]
    Wk = Wqkv[N_HEAD * HD:(N_HEAD + N_KV) * HD]
    Wv = Wqkv[(N_HEAD + N_KV) * HD:]

    in_maps = []
    for c in range(N_CORES):
        g, b = c % 4, c // 4
        xTb = np.ascontiguousarray(x[b].T.astype(NP_DT))
        wq = Wq[g * QW:(g + 1) * QW]
        wk = Wk[g * KW:(g + 1) * KW]
        wv = Wv[g * KW:(g + 1) * KW]
        wqkvT = np.ascontiguousarray(
            np.concatenate([wq, wk, wv], axis=0).T.astype(NP_DT))
        wpT = np.ascontiguousarray(Wproj[g * EW:(g + 1) * EW].T.astype(NP_DT))
        in_maps.append({"xT": xTb, "wqkvT": wqkvT, "wpT": wpT,
                        "cosf": cosf, "sinf": sinf})
    return in_maps


def run(in_maps, **kw):
    nc = _get_program()
    return run_bass_kernel_spmd(nc, in_maps, core_ids=list(range(N_CORES)), **kw)


def assemble(results):
    B = 2
    out = np.empty((B, T, C), dtype=np.float32)
    for c in range(N_CORES):
        g, b = c % 4, c // 4
        out[b][:, g * EW:(g + 1) * EW] = results[c]["out"]
    return out


def kernel(x, Wqkv, Wproj, freqs_cos, freqs_sin, start_pos=None, **_ignored):
    in_maps = prepare_in_maps(x, Wqkv, Wproj, freqs_cos, freqs_sin)
    res = run(in_maps)
    return assemble(res.results)
